# revision 14
# baseline (speedup 1.0000x reference)
"""Multi-head self-attention prefill (B=2, S=2048, E=2048, H=16, D=128) on 8 trn2 cores.

Sharding: core c -> batch b = c//4, head-group g = c%4 (heads 4g..4g+3).
Each core computes q/k/v projections for its 4 heads (column shard of Wq/Wk/Wv),
causal attention with RoPE, and a partial output projection (row shard of Wo).
Host sums the 4 partials per batch (all-reduce equivalent) and stacks batches.

v2 design (bf16 everywhere):
 - all matmuls in bf16 (full PE rate, FWL weight loads), PSUM accumulation f32
 - x transposed by the DMA XBAR engine (dma_start_transpose) -> no PE transposes
 - scores computed pre-transposed ([k, q] layout) so attention needs NO PE
   transposes and no PSUM->SBUF attn copies; softmax row-sums z are computed
   with an all-ones stationary matmul (broadcast over partitions), and the
   1/z normalization is applied once to the small ctx^T tile
 - weights resident in SBUF (loaded once), output written as bf16
"""
import sys
sys.path.insert(0, "/opt/trn_rl_repo")
import numpy as np

import concourse.bass as bass
import concourse.mybir as mybir
import concourse.tile as tile
from concourse import bacc
from concourse.bass import ds, ts
from concourse.bass_utils import run_bass_kernel_spmd

S = 2048          # sequence length (per batch)
E = 2048          # embedding dim
H = 16            # total heads
D = 128           # head dim
HG = 4            # heads per core
DG = HG * D       # 512: per-core projection width
NE = E // 128     # 16 contraction chunks
NTB = 4           # token blocks of 512
TB = S // NTB     # 512
NTT = S // 128    # 16 token tiles of 128
NQC = 4           # q-chunks of 512
ROPE_BASE = 10000.0
MASK_VAL = -1e30

f32 = mybir.dt.float32
bf16 = mybir.dt.bfloat16

_CACHE = {}


def build():
    nc = bacc.Bacc(None)
    x_in = nc.dram_tensor("x", [S, E], bf16, kind="ExternalInput")
    wq_in = nc.dram_tensor("wq", [E, DG], bf16, kind="ExternalInput")
    wk_in = nc.dram_tensor("wk", [E, DG], bf16, kind="ExternalInput")
    wv_in = nc.dram_tensor("wv", [E, DG], bf16, kind="ExternalInput")
    wo_in = nc.dram_tensor("wo", [DG, E], bf16, kind="ExternalInput")
    cos_in = nc.dram_tensor("cosT", [128, S], bf16, kind="ExternalInput")
    sin_in = nc.dram_tensor("sinT", [128, S], bf16, kind="ExternalInput")
    out_d = nc.dram_tensor("out", [S, E], bf16, kind="ExternalOutput")

    with tile.TileContext(nc) as tc:
        with tc.tile_pool(name="persist", bufs=1) as pp:
            # persistent across phases
            qT = [pp.tile([128, S], bf16, tag=f"qT{h}", name=f"qT{h}") for h in range(HG)]
            kT = [pp.tile([128, S], bf16, tag=f"kT{h}", name=f"kT{h}") for h in range(HG)]
            v_sb = [pp.tile([128, DG], bf16, tag=f"v{tt}", name=f"v{tt}") for tt in range(NTT)]
            wo_sb = [pp.tile([128, E], bf16, tag=f"wo{h}", name=f"wo{h}") for h in range(HG)]
            cosT = pp.tile([128, S], bf16, tag="cos")
            sinT = pp.tile([128, S], bf16, tag="sin")
            ones_sb = pp.tile([128, 128], bf16, tag="ones")
            nc.gpsimd.memset(ones_sb[:], 1.0)
            # transposed causal mask for [k(part), q(free)] diag blocks:
            # keep (0.0) where q >= k, else MASK_VAL
            maskT = pp.tile([128, 128], f32, tag="maskT")
            nc.gpsimd.memset(maskT[:], 0.0)
            nc.gpsimd.affine_select(
                out=maskT[:], in_=maskT[:],
                compare_op=mybir.AluOpType.is_ge, fill=MASK_VAL,
                base=0, pattern=[[1, 128]], channel_multiplier=-1)

            # ---------------- Phase A: xT via DMA, projections, RoPE ----------------
            with tc.tile_pool(name="phA", bufs=1) as pa, \
                 tc.tile_pool(name="phA2", bufs=3) as pa2, \
                 tc.tile_pool(name="psA", bufs=8, space="PSUM") as psA:
                wq_sb = [pa.tile([128, DG], bf16, tag=f"wq{e}", name=f"wq{e}") for e in range(NE)]
                wk_sb = [pa.tile([128, DG], bf16, tag=f"wk{e}", name=f"wk{e}") for e in range(NE)]
                wv_sb = [pa.tile([128, DG], bf16, tag=f"wv{e}", name=f"wv{e}") for e in range(NE)]
                xT = [pa.tile([128, S], bf16, tag=f"xT{e}", name=f"xT{e}") for e in range(NE)]

                # All DMAs share one global in-flight window in emission
                # order, so interleave each x-column transpose with the
                # weight chunks needed alongside it.
                nc.scalar.dma_start(out=cosT[:], in_=cos_in[:])
                nc.scalar.dma_start(out=sinT[:], in_=sin_in[:])
                for e in range(NE):
                    nc.sync.dma_start_transpose(out=xT[e][:], in_=x_in[:, ts(e, 128)])
                    nc.sync.dma_start(out=wq_sb[e][:], in_=wq_in[ts(e, 128), :])
                    nc.sync.dma_start(out=wk_sb[e][:], in_=wk_in[ts(e, 128), :])
                for e in range(NE):
                    nc.sync.dma_start(out=wv_sb[e][:], in_=wv_in[ts(e, 128), :])
                for h in range(HG):
                    nc.sync.dma_start(out=wo_sb[h][:], in_=wo_in[ts(h, 128), :])

                def rope_drain(ps, dstT, h, tb):
                    """RoPE: dst = stage*cos + swap(stage)*sin."""
                    stg = pa2.tile([128, TB], bf16, tag="stg")
                    nc.scalar.copy(stg[:], ps[:])
                    swp = pa2.tile([128, TB], bf16, tag="swp")
                    nc.scalar.dma_start(out=swp[0:64, :], in_=stg[64:128, :])
                    nc.scalar.dma_start(out=swp[64:128, :], in_=stg[0:64, :])
                    t1 = pa2.tile([128, TB], f32, tag="t1")
                    nc.vector.tensor_mul(t1[:], stg[:], cosT[:, ts(tb, TB)])
                    t2 = pa2.tile([128, TB], f32, tag="t2")
                    nc.vector.tensor_mul(t2[:], swp[:], sinT[:, ts(tb, TB)])
                    nc.vector.tensor_add(dstT[h][:, ts(tb, TB)], t1[:], t2[:])

                # h=0 q+k: accumulate in e-arrival order across all 8 banks,
                # so the PE computes while the x transposes stream in.
                ps8 = [psA.tile([128, TB], f32, tag="pA", name=f"p8{i}")
                       for i in range(8)]
                for e in range(NE):
                    for i in range(8):
                        w_sb = wq_sb if i < 4 else wk_sb
                        nc.tensor.matmul(ps8[i][:], w_sb[e][:, ts(0, 128)],
                                         xT[e][:, ts(i % 4, TB)],
                                         start=(e == 0), stop=(e == NE - 1))
                for i in range(8):
                    rope_drain(ps8[i], qT if i < 4 else kT, 0, i % 4)

                def proj_rope(w_sb, dstT, h):
                    for tb in range(NTB):
                        ps = psA.tile([128, TB], f32, tag="pA")
                        for e in range(NE):
                            nc.tensor.matmul(ps[:], w_sb[e][:, ts(h, 128)],
                                             xT[e][:, ts(tb, TB)],
                                             start=(e == 0), stop=(e == NE - 1))
                        rope_drain(ps, dstT, h, tb)

                for h in range(1, HG):
                    proj_rope(wq_sb, qT, h)
                    proj_rope(wk_sb, kT, h)
                for tt in range(NTT):
                    ps = psA.tile([128, DG], f32, tag="pA")
                    for e in range(NE):
                        nc.tensor.matmul(ps[:], xT[e][:, ts(tt, 128)],
                                         wv_sb[e][:],
                                         start=(e == 0), stop=(e == NE - 1))
                    nc.scalar.copy(v_sb[tt][:], ps[:])

            # ---------------- Phase B: attention + output projection ----------------
            with tc.tile_pool(name="phB", bufs=2) as pb2, \
                 tc.tile_pool(name="psS", bufs=2, space="PSUM") as psS, \
                 tc.tile_pool(name="psZ", bufs=2, space="PSUM") as psZ, \
                 tc.tile_pool(name="psC", bufs=2, space="PSUM") as psC, \
                 tc.tile_pool(name="psO", bufs=2, space="PSUM") as psO:
                for qc in range(NQC):
                    ctxs = {}
                    for h in range(HG):
                        ctx_ps = psC.tile([128, 512], f32, tag="ctx")
                        z_ps = psZ.tile([128, 512], f32, tag="z")
                        nkt = 4 * qc + 4
                        for kt in range(nkt):
                            j = kt - 4 * qc
                            off = max(0, j) * 128
                            w = 512 - off
                            sc = psS.tile([128, 512], f32, tag="sc")
                            nc.tensor.matmul(sc[:, ds(off, w)],
                                             kT[h][:, ts(kt, 128)],
                                             qT[h][:, ds(qc * 512 + off, w)],
                                             start=True, stop=True)
                            if j >= 0:
                                # diagonal block: triangular causal mask
                                nc.vector.tensor_add(sc[:, ds(off, 128)],
                                                     sc[:, ds(off, 128)], maskT[:])
                            ex = pb2.tile([128, 512], bf16, tag="ex", bufs=3)
                            nc.scalar.activation(ex[:, ds(off, w)], sc[:, ds(off, w)],
                                                 mybir.ActivationFunctionType.Exp)
                            nc.tensor.matmul(z_ps[:, ds(off, w)], ones_sb[:],
                                             ex[:, ds(off, w)],
                                             start=(kt == 0), stop=(kt == nkt - 1))
                            nc.tensor.matmul(ctx_ps[:, ds(off, w)],
                                             v_sb[kt][:, ts(h, 128)],
                                             ex[:, ds(off, w)],
                                             start=(kt == 0), stop=(kt == nkt - 1))
                        rz = pb2.tile([128, 512], f32, tag="rz")
                        nc.vector.reciprocal_approx_fast(out=rz[:], in_=z_ps[:])
                        ct = pb2.tile([128, 512], bf16, tag=f"ctxT{h}")
                        nc.vector.tensor_mul(ct[:], ctx_ps[:], rz[:])
                        ctxs[h] = ct
                    # output projection for this q-chunk
                    for e4 in range(4):
                        for t4 in range(4):
                            po = psO.tile([128, 512], f32, tag="po")
                            for h in range(HG):
                                nc.tensor.matmul(po[:], ctxs[h][:, ts(t4, 128)],
                                                 wo_sb[h][:, ts(e4, 512)],
                                                 start=(h == 0), stop=(h == HG - 1))
                            ob = pb2.tile([128, 512], bf16, tag="ob", bufs=3)
                            (nc.scalar.copy if t4 % 2 else nc.vector.tensor_copy)(
                                ob[:], po[:])
                            nc.sync.dma_start(
                                out=out_d[ds(qc * 512 + t4 * 128, 128), ts(e4, 512)],
                                in_=ob[:])
    nc.finalize()
    return nc


def _host_tables():
    half = D // 2
    inv = 1.0 / (ROPE_BASE ** (np.arange(half, dtype=np.float64) * 2.0 / D))
    ang = np.arange(S, dtype=np.float64)[None, :] * inv[:, None]   # [64, S]
    cos = np.cos(ang)
    sin = np.sin(ang)
    cosT = np.concatenate([cos, cos], axis=0)                      # [128, S]
    sinT = np.concatenate([-sin, sin], axis=0)                     # [128, S]
    return cosT, sinT


def kernel(x, start_pos, Wq, Wk, Wv, Wo):
    import ml_dtypes
    bf = ml_dtypes.bfloat16
    x = np.asarray(x, dtype=np.float32)
    Wq = np.asarray(Wq, dtype=np.float32)
    Wk = np.asarray(Wk, dtype=np.float32)
    Wv = np.asarray(Wv, dtype=np.float32)
    Wo = np.asarray(Wo, dtype=np.float32)
    B = x.shape[0]
    assert x.shape == (B, S, E) and B == 2

    cosT, sinT = _host_tables()
    cosT = cosT.astype(bf)
    sinT = sinT.astype(bf)
    perm = np.concatenate([np.arange(0, D, 2), np.arange(1, D, 2)])
    scale = 1.0 / np.sqrt(D)

    in_maps = []
    for c in range(8):
        b, g = c // 4, c % 4
        cols = slice(DG * g, DG * g + DG)
        wq = (Wq[:, cols] * scale).reshape(E, HG, D)[:, :, perm].reshape(E, DG)
        wk = Wk[:, cols].reshape(E, HG, D)[:, :, perm].reshape(E, DG)
        in_maps.append({
            "x": np.ascontiguousarray(x[b]).astype(bf),
            "wq": np.ascontiguousarray(wq).astype(bf),
            "wk": np.ascontiguousarray(wk).astype(bf),
            "wv": np.ascontiguousarray(Wv[:, cols]).astype(bf),
            "wo": np.ascontiguousarray(Wo[cols, :]).astype(bf),
            "cosT": cosT,
            "sinT": sinT,
        })

    if "nc" not in _CACHE:
        _CACHE["nc"] = build()
    nc = _CACHE["nc"]
    _CACHE["in_maps"] = in_maps
    res = run_bass_kernel_spmd(nc, in_maps, list(range(8)))
    parts = [np.asarray(res.results[c]["out"], dtype=np.float32) for c in range(8)]
    out = np.stack([
        parts[0] + parts[1] + parts[2] + parts[3],
        parts[4] + parts[5] + parts[6] + parts[7],
    ]).astype(np.float32)
    return out


# revision 15
# speedup vs baseline: 1.0285x; 1.0285x over previous
"""Multi-head self-attention prefill (B=2, S=2048, E=2048, H=16, D=128) on 8 trn2 cores.

Sharding: core c -> batch b = c//4, head-group g = c%4 (heads 4g..4g+3).
Each core computes q/k/v projections for its 4 heads (column shard of Wq/Wk/Wv),
causal attention with RoPE, and a partial output projection (row shard of Wo).
Host sums the 4 partials per batch (all-reduce equivalent) and stacks batches.

v2 design (bf16 everywhere):
 - all matmuls in bf16 (full PE rate, FWL weight loads), PSUM accumulation f32
 - x transposed by the DMA XBAR engine (dma_start_transpose) -> no PE transposes
 - scores computed pre-transposed ([k, q] layout) so attention needs NO PE
   transposes and no PSUM->SBUF attn copies; softmax row-sums z are computed
   with an all-ones stationary matmul (broadcast over partitions), and the
   1/z normalization is applied once to the small ctx^T tile
 - weights resident in SBUF (loaded once), output written as bf16
"""
import sys
sys.path.insert(0, "/opt/trn_rl_repo")
import numpy as np

import concourse.bass as bass
import concourse.mybir as mybir
import concourse.tile as tile
from concourse import bacc
from concourse.bass import ds, ts
from concourse.bass_utils import run_bass_kernel_spmd

S = 2048          # sequence length (per batch)
E = 2048          # embedding dim
H = 16            # total heads
D = 128           # head dim
HG = 4            # heads per core
DG = HG * D       # 512: per-core projection width
NE = E // 128     # 16 contraction chunks
NTB = 4           # token blocks of 512
TB = S // NTB     # 512
NTT = S // 128    # 16 token tiles of 128
NQC = 4           # q-chunks of 512
ROPE_BASE = 10000.0
MASK_VAL = -1e30

f32 = mybir.dt.float32
bf16 = mybir.dt.bfloat16

_CACHE = {}


def build():
    nc = bacc.Bacc(None)
    x_in = nc.dram_tensor("x", [S, E], bf16, kind="ExternalInput")
    wq_in = nc.dram_tensor("wq", [E, DG], bf16, kind="ExternalInput")
    wk_in = nc.dram_tensor("wk", [E, DG], bf16, kind="ExternalInput")
    wv_in = nc.dram_tensor("wv", [E, DG], bf16, kind="ExternalInput")
    wo_in = nc.dram_tensor("wo", [DG, E], bf16, kind="ExternalInput")
    cos_in = nc.dram_tensor("cosT", [128, S], bf16, kind="ExternalInput")
    sin_in = nc.dram_tensor("sinT", [128, S], bf16, kind="ExternalInput")
    out_d = nc.dram_tensor("out", [S, E], bf16, kind="ExternalOutput")

    with tile.TileContext(nc) as tc:
        with tc.tile_pool(name="persist", bufs=1) as pp:
            # persistent across phases
            qT = [pp.tile([128, S], bf16, tag=f"qT{h}", name=f"qT{h}") for h in range(HG)]
            kT = [pp.tile([128, S], bf16, tag=f"kT{h}", name=f"kT{h}") for h in range(HG)]
            v_sb = [pp.tile([128, DG], bf16, tag=f"v{tt}", name=f"v{tt}") for tt in range(NTT)]
            wo_sb = [pp.tile([128, E], bf16, tag=f"wo{h}", name=f"wo{h}") for h in range(HG)]
            cosT = pp.tile([128, S], bf16, tag="cos")
            sinT = pp.tile([128, S], bf16, tag="sin")
            ones_sb = pp.tile([128, 128], bf16, tag="ones")
            nc.gpsimd.memset(ones_sb[:], 1.0)
            # transposed causal mask for [k(part), q(free)] diag blocks:
            # keep (0.0) where q >= k, else MASK_VAL
            maskT = pp.tile([128, 128], f32, tag="maskT")
            nc.gpsimd.memset(maskT[:], 0.0)
            nc.gpsimd.affine_select(
                out=maskT[:], in_=maskT[:],
                compare_op=mybir.AluOpType.is_ge, fill=MASK_VAL,
                base=0, pattern=[[1, 128]], channel_multiplier=-1)

            # ---------------- Phase A: xT via DMA, projections, RoPE ----------------
            with tc.tile_pool(name="phA", bufs=1) as pa, \
                 tc.tile_pool(name="phA2", bufs=3) as pa2, \
                 tc.tile_pool(name="psA", bufs=8, space="PSUM") as psA:
                wq_sb = [pa.tile([128, DG], bf16, tag=f"wq{e}", name=f"wq{e}") for e in range(NE)]
                wk_sb = [pa.tile([128, DG], bf16, tag=f"wk{e}", name=f"wk{e}") for e in range(NE)]
                wv_sb = [pa.tile([128, DG], bf16, tag=f"wv{e}", name=f"wv{e}") for e in range(NE)]
                xT = [pa.tile([128, S], bf16, tag=f"xT{e}", name=f"xT{e}") for e in range(NE)]

                # All DMAs share one global in-flight window in emission
                # order, so interleave each x-column transpose with the
                # weight chunks needed alongside it.
                nc.scalar.dma_start(out=cosT[:], in_=cos_in[:])
                nc.scalar.dma_start(out=sinT[:], in_=sin_in[:])
                for e in range(NE):
                    nc.sync.dma_start_transpose(out=xT[e][:], in_=x_in[:, ts(e, 128)])
                    nc.sync.dma_start(out=wq_sb[e][:], in_=wq_in[ts(e, 128), :])
                    nc.sync.dma_start(out=wk_sb[e][:], in_=wk_in[ts(e, 128), :])
                def rope_drain(ps, dstT, h, tb):
                    """RoPE: dst = stage*cos + swap(stage)*sin."""
                    stg = pa2.tile([128, TB], bf16, tag="stg")
                    nc.scalar.copy(stg[:], ps[:])
                    swp = pa2.tile([128, TB], bf16, tag="swp")
                    nc.scalar.dma_start(out=swp[0:64, :], in_=stg[64:128, :])
                    nc.scalar.dma_start(out=swp[64:128, :], in_=stg[0:64, :])
                    t1 = pa2.tile([128, TB], f32, tag="t1")
                    nc.vector.tensor_mul(t1[:], stg[:], cosT[:, ts(tb, TB)])
                    t2 = pa2.tile([128, TB], f32, tag="t2")
                    nc.vector.tensor_mul(t2[:], swp[:], sinT[:, ts(tb, TB)])
                    nc.vector.tensor_add(dstT[h][:, ts(tb, TB)], t1[:], t2[:])

                # h=0 q+k: accumulate in e-arrival order across all 8 banks,
                # so the PE computes while the x transposes stream in.
                ps8 = [psA.tile([128, TB], f32, tag="pA", name=f"p8{i}")
                       for i in range(8)]
                for e in range(NE):
                    for i in range(8):
                        w_sb = wq_sb if i < 4 else wk_sb
                        nc.tensor.matmul(ps8[i][:], w_sb[e][:, ts(0, 128)],
                                         xT[e][:, ts(i % 4, TB)],
                                         start=(e == 0), stop=(e == NE - 1))
                for i in range(8):
                    rope_drain(ps8[i], qT if i < 4 else kT, 0, i % 4)

                def proj_rope(w_sb, dstT, h):
                    for tb in range(NTB):
                        ps = psA.tile([128, TB], f32, tag="pA")
                        for e in range(NE):
                            nc.tensor.matmul(ps[:], w_sb[e][:, ts(h, 128)],
                                             xT[e][:, ts(tb, TB)],
                                             start=(e == 0), stop=(e == NE - 1))
                        rope_drain(ps, dstT, h, tb)

                for h in range(1, HG):
                    proj_rope(wq_sb, qT, h)
                    proj_rope(wk_sb, kT, h)
                # emit wv/wo loads only now: DMAs complete in emission order
                # (global in-flight window), so earlier emission would stall
                # the RoPE swap DMAs behind them
                for e in range(NE):
                    nc.sync.dma_start(out=wv_sb[e][:], in_=wv_in[ts(e, 128), :])
                for h in range(HG):
                    nc.sync.dma_start(out=wo_sb[h][:], in_=wo_in[ts(h, 128), :])
                for tt in range(NTT):
                    ps = psA.tile([128, DG], f32, tag="pA")
                    for e in range(NE):
                        nc.tensor.matmul(ps[:], xT[e][:, ts(tt, 128)],
                                         wv_sb[e][:],
                                         start=(e == 0), stop=(e == NE - 1))
                    nc.scalar.copy(v_sb[tt][:], ps[:])

            # ---------------- Phase B: attention + output projection ----------------
            with tc.tile_pool(name="phB", bufs=2) as pb2, \
                 tc.tile_pool(name="psS", bufs=2, space="PSUM") as psS, \
                 tc.tile_pool(name="psZ", bufs=2, space="PSUM") as psZ, \
                 tc.tile_pool(name="psC", bufs=2, space="PSUM") as psC, \
                 tc.tile_pool(name="psO", bufs=2, space="PSUM") as psO:
                for qc in range(NQC):
                    ctxs = {}
                    for h in range(HG):
                        ctx_ps = psC.tile([128, 512], f32, tag="ctx")
                        z_ps = psZ.tile([128, 512], f32, tag="z")
                        nkt = 4 * qc + 4
                        for kt in range(nkt):
                            j = kt - 4 * qc
                            off = max(0, j) * 128
                            w = 512 - off
                            sc = psS.tile([128, 512], f32, tag="sc")
                            nc.tensor.matmul(sc[:, ds(off, w)],
                                             kT[h][:, ts(kt, 128)],
                                             qT[h][:, ds(qc * 512 + off, w)],
                                             start=True, stop=True)
                            if j >= 0:
                                # diagonal block: triangular causal mask
                                nc.vector.tensor_add(sc[:, ds(off, 128)],
                                                     sc[:, ds(off, 128)], maskT[:])
                            ex = pb2.tile([128, 512], bf16, tag="ex", bufs=3)
                            nc.scalar.activation(ex[:, ds(off, w)], sc[:, ds(off, w)],
                                                 mybir.ActivationFunctionType.Exp)
                            nc.tensor.matmul(z_ps[:, ds(off, w)], ones_sb[:],
                                             ex[:, ds(off, w)],
                                             start=(kt == 0), stop=(kt == nkt - 1))
                            nc.tensor.matmul(ctx_ps[:, ds(off, w)],
                                             v_sb[kt][:, ts(h, 128)],
                                             ex[:, ds(off, w)],
                                             start=(kt == 0), stop=(kt == nkt - 1))
                        rz = pb2.tile([128, 512], f32, tag="rz")
                        nc.vector.reciprocal_approx_fast(out=rz[:], in_=z_ps[:])
                        ct = pb2.tile([128, 512], bf16, tag=f"ctxT{h}")
                        nc.vector.tensor_mul(ct[:], ctx_ps[:], rz[:])
                        ctxs[h] = ct
                    # output projection for this q-chunk
                    for e4 in range(4):
                        for t4 in range(4):
                            po = psO.tile([128, 512], f32, tag="po")
                            for h in range(HG):
                                nc.tensor.matmul(po[:], ctxs[h][:, ts(t4, 128)],
                                                 wo_sb[h][:, ts(e4, 512)],
                                                 start=(h == 0), stop=(h == HG - 1))
                            ob = pb2.tile([128, 512], bf16, tag="ob", bufs=3)
                            (nc.scalar.copy if t4 % 2 else nc.vector.tensor_copy)(
                                ob[:], po[:])
                            nc.sync.dma_start(
                                out=out_d[ds(qc * 512 + t4 * 128, 128), ts(e4, 512)],
                                in_=ob[:])
    nc.finalize()
    return nc


def _host_tables():
    half = D // 2
    inv = 1.0 / (ROPE_BASE ** (np.arange(half, dtype=np.float64) * 2.0 / D))
    ang = np.arange(S, dtype=np.float64)[None, :] * inv[:, None]   # [64, S]
    cos = np.cos(ang)
    sin = np.sin(ang)
    cosT = np.concatenate([cos, cos], axis=0)                      # [128, S]
    sinT = np.concatenate([-sin, sin], axis=0)                     # [128, S]
    return cosT, sinT


def kernel(x, start_pos, Wq, Wk, Wv, Wo):
    import ml_dtypes
    bf = ml_dtypes.bfloat16
    x = np.asarray(x, dtype=np.float32)
    Wq = np.asarray(Wq, dtype=np.float32)
    Wk = np.asarray(Wk, dtype=np.float32)
    Wv = np.asarray(Wv, dtype=np.float32)
    Wo = np.asarray(Wo, dtype=np.float32)
    B = x.shape[0]
    assert x.shape == (B, S, E) and B == 2

    cosT, sinT = _host_tables()
    cosT = cosT.astype(bf)
    sinT = sinT.astype(bf)
    perm = np.concatenate([np.arange(0, D, 2), np.arange(1, D, 2)])
    scale = 1.0 / np.sqrt(D)

    in_maps = []
    for c in range(8):
        b, g = c // 4, c % 4
        cols = slice(DG * g, DG * g + DG)
        wq = (Wq[:, cols] * scale).reshape(E, HG, D)[:, :, perm].reshape(E, DG)
        wk = Wk[:, cols].reshape(E, HG, D)[:, :, perm].reshape(E, DG)
        in_maps.append({
            "x": np.ascontiguousarray(x[b]).astype(bf),
            "wq": np.ascontiguousarray(wq).astype(bf),
            "wk": np.ascontiguousarray(wk).astype(bf),
            "wv": np.ascontiguousarray(Wv[:, cols]).astype(bf),
            "wo": np.ascontiguousarray(Wo[cols, :]).astype(bf),
            "cosT": cosT,
            "sinT": sinT,
        })

    if "nc" not in _CACHE:
        _CACHE["nc"] = build()
    nc = _CACHE["nc"]
    _CACHE["in_maps"] = in_maps
    res = run_bass_kernel_spmd(nc, in_maps, list(range(8)))
    parts = [np.asarray(res.results[c]["out"], dtype=np.float32) for c in range(8)]
    out = np.stack([
        parts[0] + parts[1] + parts[2] + parts[3],
        parts[4] + parts[5] + parts[6] + parts[7],
    ]).astype(np.float32)
    return out


# revision 16
# speedup vs baseline: 1.1911x; 1.1581x over previous
"""Multi-head self-attention prefill (B=2, S=2048, E=2048, H=16, D=128) on 8 trn2 cores.

Sharding: core c -> batch b = c//4, head-group g = c%4 (heads 4g..4g+3).
Each core computes q/k/v projections for its 4 heads (column shard of Wq/Wk/Wv),
causal attention with RoPE, and a partial output projection (row shard of Wo).
Host sums the 4 partials per batch (all-reduce equivalent) and stacks batches.

Design (bf16):
 - all matmuls bf16 (full PE rate, FWL weight loads), PSUM accumulation f32
 - x transposed by the DMA XBAR engine (dma_start_transpose): no PE transposes
 - scores computed pre-transposed ([k, q] layout): attention needs NO PE
   transposes and no PSUM->SBUF attn copies; softmax row-sums z via an
   all-ones stationary matmul (broadcast over partitions); 1/z applied once
   to the ctx^T tile (reciprocal_approx_fast)
 - phase B emits two heads' kt-streams interleaved with one-step lookahead so
   the PE never waits for the exp round-trip
 - weights resident in SBUF, output written as bf16
"""
import sys
sys.path.insert(0, "/opt/trn_rl_repo")
import numpy as np

import concourse.bass as bass
import concourse.mybir as mybir
import concourse.tile as tile
from concourse import bacc
from concourse.bass import ds, ts
from concourse.bass_utils import run_bass_kernel_spmd

S = 2048          # sequence length (per batch)
E = 2048          # embedding dim
H = 16            # total heads
D = 128           # head dim
HG = 4            # heads per core
DG = HG * D       # 512: per-core projection width
NE = E // 128     # 16 contraction chunks
NTB = 4           # token blocks of 512
TB = S // NTB     # 512
NTT = S // 128    # 16 token tiles of 128
NQC = 4           # q-chunks of 512
ROPE_BASE = 10000.0
MASK_VAL = -1e30

f32 = mybir.dt.float32
bf16 = mybir.dt.bfloat16

_CACHE = {}


def build():
    nc = bacc.Bacc(None)
    x_in = nc.dram_tensor("x", [S, E], bf16, kind="ExternalInput")
    wq_in = nc.dram_tensor("wq", [E, DG], bf16, kind="ExternalInput")
    wk_in = nc.dram_tensor("wk", [E, DG], bf16, kind="ExternalInput")
    wv_in = nc.dram_tensor("wv", [E, DG], bf16, kind="ExternalInput")
    wo_in = nc.dram_tensor("wo", [DG, E], bf16, kind="ExternalInput")
    cos_in = nc.dram_tensor("cosT", [128, S], bf16, kind="ExternalInput")
    sin_in = nc.dram_tensor("sinT", [128, S], bf16, kind="ExternalInput")
    out_d = nc.dram_tensor("out", [S, E], bf16, kind="ExternalOutput")

    with tile.TileContext(nc) as tc:
        with tc.tile_pool(name="persist", bufs=1) as pp:
            qT = [pp.tile([128, S], bf16, tag=f"qT{h}", name=f"qT{h}") for h in range(HG)]
            kT = [pp.tile([128, S], bf16, tag=f"kT{h}", name=f"kT{h}") for h in range(HG)]
            v_sb = [pp.tile([128, DG], bf16, tag=f"v{tt}", name=f"v{tt}") for tt in range(NTT)]
            wo_sb = [pp.tile([128, E], bf16, tag=f"wo{h}", name=f"wo{h}") for h in range(HG)]
            cosT = pp.tile([128, S], bf16, tag="cos")
            sinT = pp.tile([128, S], bf16, tag="sin")
            ones_sb = pp.tile([128, 128], bf16, tag="ones")
            nc.gpsimd.memset(ones_sb[:], 1.0)
            # transposed causal mask for [k(part), q(free)] diag blocks:
            # keep (0.0) where q >= k, else MASK_VAL
            maskT = pp.tile([128, 128], f32, tag="maskT")
            nc.gpsimd.memset(maskT[:], 0.0)
            nc.gpsimd.affine_select(
                out=maskT[:], in_=maskT[:],
                compare_op=mybir.AluOpType.is_ge, fill=MASK_VAL,
                base=0, pattern=[[1, 128]], channel_multiplier=-1)

            # ---------------- Phase A: xT via DMA, projections, RoPE ----------------
            with tc.tile_pool(name="phA", bufs=1) as pa, \
                 tc.tile_pool(name="phA2", bufs=3) as pa2, \
                 tc.tile_pool(name="psA", bufs=3, space="PSUM") as psA:
                wq_sb = [pa.tile([128, DG], bf16, tag=f"wq{e}", name=f"wq{e}") for e in range(NE)]
                wk_sb = [pa.tile([128, DG], bf16, tag=f"wk{e}", name=f"wk{e}") for e in range(NE)]
                wv_sb = [pa.tile([128, DG], bf16, tag=f"wv{e}", name=f"wv{e}") for e in range(NE)]
                xT = [pa.tile([128, S], bf16, tag=f"xT{e}", name=f"xT{e}") for e in range(NE)]
                # x transposes gate all compute: split across both hwdge queues
                for e in range(NE):
                    eng = nc.sync if e % 2 == 0 else nc.scalar
                    eng.dma_start_transpose(out=xT[e][:], in_=x_in[:, ts(e, 128)])
                # weights, in first-use order, one matrix per queue
                for e in range(NE):
                    nc.sync.dma_start(out=wq_sb[e][:], in_=wq_in[ts(e, 128), :])
                    nc.scalar.dma_start(out=wk_sb[e][:], in_=wk_in[ts(e, 128), :])
                nc.sync.dma_start(out=cosT[:], in_=cos_in[:])
                nc.sync.dma_start(out=sinT[:], in_=sin_in[:])
                for e in range(NE):
                    nc.scalar.dma_start(out=wv_sb[e][:], in_=wv_in[ts(e, 128), :])
                for h in range(HG):
                    nc.scalar.dma_start(out=wo_sb[h][:], in_=wo_in[ts(h, 128), :])

                def proj_rope(w_sb, dstT, h):
                    """q or k projection for head h + RoPE into dstT[h]."""
                    for tb in range(NTB):
                        ps = psA.tile([128, TB], f32, tag="pA")
                        for e in range(NE):
                            nc.tensor.matmul(ps[:], w_sb[e][:, ts(h, 128)],
                                             xT[e][:, ts(tb, TB)],
                                             start=(e == 0), stop=(e == NE - 1))
                        # RoPE: dst = stage*cos + swap(stage)*sin
                        stg = pa2.tile([128, TB], bf16, tag="stg")
                        nc.scalar.copy(stg[:], ps[:])
                        swp = pa2.tile([128, TB], bf16, tag="swp")
                        nc.sync.dma_start(out=swp[0:64, :], in_=stg[64:128, :])
                        nc.sync.dma_start(out=swp[64:128, :], in_=stg[0:64, :])
                        t1 = pa2.tile([128, TB], f32, tag="t1")
                        nc.vector.tensor_mul(t1[:], stg[:], cosT[:, ts(tb, TB)])
                        t2 = pa2.tile([128, TB], f32, tag="t2")
                        nc.gpsimd.tensor_mul(t2[:], swp[:], sinT[:, ts(tb, TB)])
                        nc.vector.tensor_add(dstT[h][:, ts(tb, TB)], t1[:], t2[:])

                for h in range(HG):
                    proj_rope(wq_sb, qT, h)
                    proj_rope(wk_sb, kT, h)
                    # v for token tiles 4h..4h+3 (so early q-chunks unblock soon)
                    for tt in range(4 * h, 4 * h + 4):
                        ps = psA.tile([128, DG], f32, tag="pA")
                        for e in range(NE):
                            nc.tensor.matmul(ps[:], xT[e][:, ts(tt, 128)],
                                             wv_sb[e][:],
                                             start=(e == 0), stop=(e == NE - 1))
                        nc.scalar.copy(v_sb[tt][:], ps[:])

            # ---------------- Phase B: attention + output projection ----------------
            with tc.tile_pool(name="phB", bufs=2) as pb2, \
                 tc.tile_pool(name="psS", bufs=2, space="PSUM") as psS, \
                 tc.tile_pool(name="psZ", bufs=2, space="PSUM") as psZ, \
                 tc.tile_pool(name="psC", bufs=2, space="PSUM") as psC, \
                 tc.tile_pool(name="psO", bufs=2, space="PSUM") as psO:

                ctxs = {}

                def attn_steps(h, qc):
                    """Emit-closures for head h, q-chunk qc: per kt a 'sc' step
                    (scores+mask+exp) and a 'zc' step (z & ctx matmuls), plus a
                    final drain. Scheduled sc(0), sc(1), zc(0), sc(2), zc(1),
                    ..., zc(last), drain: one-step lookahead so the PE has
                    queued work while exp(kt) round-trips on Act."""
                    nkt = 4 * qc + 4
                    ctx_ps = psC.tile([128, 512], f32, tag="ctx",
                                      name=f"ctx{h}_{qc}")
                    z_ps = psZ.tile([128, 512], f32, tag="z", name=f"z{h}_{qc}")
                    exs = {}

                    def sc_step(kt):
                        def emit():
                            j = kt - 4 * qc
                            off = max(0, j) * 128
                            w = 512 - off
                            sc = psS.tile([128, 512], f32, tag="sc", name="sc")
                            nc.tensor.matmul(sc[:, ds(off, w)],
                                             kT[h][:, ts(kt, 128)],
                                             qT[h][:, ds(qc * 512 + off, w)],
                                             start=True, stop=True)
                            if j >= 0:
                                nc.vector.tensor_add(sc[:, ds(off, 128)],
                                                     sc[:, ds(off, 128)], maskT[:])
                            ex = pb2.tile([128, 512], bf16, tag="ex", name="ex",
                                          bufs=4)
                            nc.scalar.activation(ex[:, ds(off, w)],
                                                 sc[:, ds(off, w)],
                                                 mybir.ActivationFunctionType.Exp)
                            exs[kt] = (ex, off, w)
                        return emit

                    def zc_step(kt):
                        def emit():
                            ex, off, w = exs.pop(kt)
                            nc.tensor.matmul(z_ps[:, ds(off, w)], ones_sb[:],
                                             ex[:, ds(off, w)],
                                             start=(kt == 0), stop=(kt == nkt - 1))
                            nc.tensor.matmul(ctx_ps[:, ds(off, w)],
                                             v_sb[kt][:, ts(h, 128)],
                                             ex[:, ds(off, w)],
                                             start=(kt == 0), stop=(kt == nkt - 1))
                        return emit

                    def drain():
                        rz = pb2.tile([128, 512], f32, tag="rz", name="rz")
                        nc.vector.reciprocal_approx_fast(out=rz[:], in_=z_ps[:])
                        ct = pb2.tile([128, 512], bf16, tag=f"ctxT{h}",
                                      name=f"ctxT{h}")
                        nc.vector.tensor_mul(ct[:], ctx_ps[:], rz[:])
                        ctxs[h] = ct

                    steps = [sc_step(0)]
                    for kt in range(1, nkt):
                        steps.append(sc_step(kt))
                        steps.append(zc_step(kt - 1))
                    steps.append(zc_step(nkt - 1))
                    steps.append(drain)
                    return steps

                def interleave(a, b):
                    out = []
                    for i in range(max(len(a), len(b))):
                        if i < len(a):
                            out.append(a[i])
                        if i < len(b):
                            out.append(b[i])
                    return out

                for qc in range(NQC):
                    for ha in (0, 2):
                        for step in interleave(attn_steps(ha, qc),
                                               attn_steps(ha + 1, qc)):
                            step()
                    # output projection for this q-chunk
                    for e4 in range(4):
                        for t4 in range(4):
                            po = psO.tile([128, 512], f32, tag="po", name="po")
                            for h in range(HG):
                                nc.tensor.matmul(po[:], ctxs[h][:, ts(t4, 128)],
                                                 wo_sb[h][:, ts(e4, 512)],
                                                 start=(h == 0), stop=(h == HG - 1))
                            ob = pb2.tile([128, 512], bf16, tag="ob", name="ob",
                                          bufs=3)
                            (nc.scalar.copy if t4 % 2 else nc.vector.tensor_copy)(
                                ob[:], po[:])
                            nc.sync.dma_start(
                                out=out_d[ds(qc * 512 + t4 * 128, 128), ts(e4, 512)],
                                in_=ob[:])
    nc.finalize()
    return nc


def _host_tables():
    half = D // 2
    inv = 1.0 / (ROPE_BASE ** (np.arange(half, dtype=np.float64) * 2.0 / D))
    ang = np.arange(S, dtype=np.float64)[None, :] * inv[:, None]   # [64, S]
    cos = np.cos(ang)
    sin = np.sin(ang)
    cosT = np.concatenate([cos, cos], axis=0)                      # [128, S]
    sinT = np.concatenate([-sin, sin], axis=0)                     # [128, S]
    return cosT, sinT


def kernel(x, start_pos, Wq, Wk, Wv, Wo):
    import ml_dtypes
    bf = ml_dtypes.bfloat16
    x = np.asarray(x, dtype=np.float32)
    Wq = np.asarray(Wq, dtype=np.float32)
    Wk = np.asarray(Wk, dtype=np.float32)
    Wv = np.asarray(Wv, dtype=np.float32)
    Wo = np.asarray(Wo, dtype=np.float32)
    B = x.shape[0]
    assert x.shape == (B, S, E) and B == 2

    cosT, sinT = _host_tables()
    cosT = cosT.astype(bf)
    sinT = sinT.astype(bf)
    perm = np.concatenate([np.arange(0, D, 2), np.arange(1, D, 2)])
    scale = 1.0 / np.sqrt(D)

    in_maps = []
    for c in range(8):
        b, g = c // 4, c % 4
        cols = slice(DG * g, DG * g + DG)
        wq = (Wq[:, cols] * scale).reshape(E, HG, D)[:, :, perm].reshape(E, DG)
        wk = Wk[:, cols].reshape(E, HG, D)[:, :, perm].reshape(E, DG)
        in_maps.append({
            "x": np.ascontiguousarray(x[b]).astype(bf),
            "wq": np.ascontiguousarray(wq).astype(bf),
            "wk": np.ascontiguousarray(wk).astype(bf),
            "wv": np.ascontiguousarray(Wv[:, cols]).astype(bf),
            "wo": np.ascontiguousarray(Wo[cols, :]).astype(bf),
            "cosT": cosT,
            "sinT": sinT,
        })

    if "nc" not in _CACHE:
        _CACHE["nc"] = build()
    nc = _CACHE["nc"]
    _CACHE["in_maps"] = in_maps
    res = run_bass_kernel_spmd(nc, in_maps, list(range(8)))
    parts = [np.asarray(res.results[c]["out"], dtype=np.float32) for c in range(8)]
    out = np.stack([
        parts[0] + parts[1] + parts[2] + parts[3],
        parts[4] + parts[5] + parts[6] + parts[7],
    ]).astype(np.float32)
    return out
